# revision 1
# baseline (speedup 1.0000x reference)
"""BiModalAttention Trainium2 kernel (v2).

Full-input contract: kernel(mode1, mode2) -> [S, B, 2D] float32.
mode1/mode2: [S=1024, B=32, D=1024] float32.

Reference computation per batch b (m1 = mode1[:, b, :], m2 = mode2[:, b, :]):
    C1 = m1 @ m2.T                  # [S, S]
    a1 = softmax_rows(C1) @ m2 * m1
    a2 = softmax_rows(C1.T) @ m1 * m2
    out[:, b, :] = concat([a1, a2], -1)

Sharding: batch dim across 8 NeuronCores (4 batch elements per core).

Per-core structure (per batch element):
  A. C1 = m1T.T @ m2T in fp32r (d-major layout via casting DMAs). fp32r
     runs as a single fp32_mode=HIGH pass (~2 cyc/row) with a mandatory
     per-matmul weight reload, so matmul count is what matters: C2 = C1.T is
     produced by PE transposes of the C1 strips (4x cheaper than a second
     fp32r matmul). C1 evacuated on ScalarE; negated row-max rm1 on VectorE.
  B. rm1 broadcast across partitions (RM1B[t,s] = -rm1[s]): DVE free-dim
     broadcast of the [P,1] column + PE transpose.
  C. C2 PSUM groups: negated row-max partials (pre-shift) -> rm2; evacuation
     fused with "+(-rm1[s])" on DVE -> epre; ACT exp -> E1T strips (bf16).
  D. E2T = exp(C1 + (-rm2[t] broadcast)) via DVE add + ACT exp -> bf16.
  E. Softmax denominators without extra matmuls: Z1[s] / Z2[t] via ACT
     exp-accumulate passes over the C1 / raw-C2 strips with the per-partition
     negated row-max as bias (all exponents <= 0, so no overflow; a
     factorized exp(rm1-rm2) trick overflows fp32 on this data).
  F. AV matmuls in bf16, 512-wide d-chunks: o1 = E1T.T @ m2chunk,
     o2 = E2T.T @ m1chunk. Evacuation fused as one DVE scalar_tensor_tensor:
     out = (psum * (1/Z)[part]) * gate, gating against the bf16 chunk of the
     other modality (same tile that feeds the AV matmul).
"""

import os
os.environ.setdefault("NEURON_RT_RESET_CORES", "1")
import time

import numpy as np

import concourse.bacc as bacc
import concourse.mybir as mybir
import concourse.tile as tile
from concourse.masks import make_identity
from concourse.bass_utils import run_bass_kernel_spmd

S = 1024
D = 1024
B = 32
N_CORES = 8
BPC = B // N_CORES          # batch elements per core
P = 128                     # partitions
NK = S // P                 # contraction tiles (8)
NI = S // P                 # s tiles (8)
CW = 512                    # AV d-chunk width (bf16 matmul moving dim)
NCH = D // CW               # AV chunks (2)

f32 = mybir.dt.float32
f32r = mybir.dt.float32r
bf16 = mybir.dt.bfloat16
AX = mybir.AxisListType
ALU = mybir.AluOpType
ACTF = mybir.ActivationFunctionType


def _emit_p1(nc, sb, ps, ident, st, j, m1t, m2t):
    # ---- Phase 1: C1 scores (fp32r) ----
    m1t_sb = sb.tile([P, NK, S], f32r, tag="m1t", bufs=1, name=f"m1t_sb{j}")
    m2t_sb = sb.tile([P, NK, S], f32r, tag="m2t", bufs=1, name=f"m2t_sb{j}")
    # halved loads: the C1 k-loop can start on the first half while the
    # second half is still in flight
    for (lo, hi) in ((0, NK // 2), (NK // 2, NK)):
        nc.gpsimd.dma_start(
            out=m1t_sb[:, lo:hi, :],
            in_=m1t[j].rearrange("(k p) s -> p k s", p=P)[:, lo:hi, :])
        nc.gpsimd.dma_start(
            out=m2t_sb[:, lo:hi, :],
            in_=m2t[j].rearrange("(k p) s -> p k s", p=P)[:, lo:hi, :])

    c1 = st["c1"] = []
    rm1 = st["rm1"] = sb.tile([P, NI], f32, tag="rm1", bufs=2, name=f"rm1_{j}")
    for i in range(NI):
        c1_i = sb.tile([P, S], f32, tag="c1", bufs=NI, name=f"c1_{j}_{i}")
        c1.append(c1_i)
        for n in range(2):
            pc = ps.tile([P, 512], f32, tag="c", bufs=4, name=f"pc{j}_{i}_{n}")
            for k in range(NK):
                nc.tensor.matmul(
                    pc,
                    m1t_sb[:, k, i * P:(i + 1) * P],
                    m2t_sb[:, k, n * 512:(n + 1) * 512],
                    start=(k == 0),
                    stop=(k == NK - 1),
                )
            nc.scalar.copy(out=c1_i[:, n * 512:(n + 1) * 512], in_=pc)
        nc.vector.tensor_reduce(rm1[:, i:i + 1], c1_i, axis=AX.X,
                                op=ALU.max, negate=True)


def _keeper(nc, ps, kc, nm):
    # tiny discarded fp32r matmul: keeps the PE HAM activity window busy so
    # the clock gate stays at 8/8 through transpose/softmax phases
    pk = ps.tile([P, 512], f32, tag="av", bufs=4, name=nm)
    nc.tensor.matmul(pk, kc[:, 0:P], kc, start=True, stop=True)


def _emit_p2(nc, sb, ps, ident, kc, st, j):
    c1 = st["c1"]
    rm1 = st["rm1"]

    # ---- negated row-max partition broadcasts ----
    def _bcast_rows(rm_cols, tag, nm):
        rmb = sb.tile([P, S], f32, tag=tag, bufs=1, name=nm)
        for g in range(2):
            pt = ps.tile([P, 512], f32, tag="c", bufs=4, name=f"{nm}_pt{g}")
            for q in range(4):
                i = g * 4 + q
                xb = sb.tile([P, P], f32, tag="xb", bufs=1, name=f"{nm}_xb{i}")
                nc.vector.tensor_copy(xb, rm_cols[:, i:i + 1].broadcast_to([P, P]))
                nc.tensor.transpose(pt[:, q * P:(q + 1) * P], xb, ident)
            nc.scalar.copy(out=rmb[:, g * 512:(g + 1) * 512], in_=pt)
        return rmb

    rm1b = _bcast_rows(rm1, "rm1b", f"rm1b_{j}")

    # ---- C2 strips via PE transpose -> rm2, Z2, E1T = exp(C2 - rm1[s]) ----
    e1 = st["e1"] = []
    rm2p = sb.tile([P, 2 * NK], f32, tag="rm2p", bufs=2, name=f"rm2p_{j}")
    rm2 = sb.tile([P, NK], f32, tag="rm2", bufs=2, name=f"rm2_{j}")
    z2p = sb.tile([P, 2 * NK], f32, tag="z2p", bufs=2, name=f"z2p_{j}")
    z2 = sb.tile([P, NK], f32, tag="z2", bufs=2, name=f"z2_{j}")
    for t in range(NK):
        e1_t = sb.tile([P, S], bf16, tag="e1", bufs=NK + 2, name=f"e1_{j}_{t}")
        e1.append(e1_t)
        epre = sb.tile([P, S], f32, tag="h", bufs=2, name=f"epre1_{j}_{t}")
        pts = []
        for g in range(2):
            pt = ps.tile([P, 512], f32, tag="c", bufs=4, name=f"pc2_{j}_{t}_{g}")
            pts.append(pt)
            for q in range(4):
                i = g * 4 + q
                nc.tensor.transpose(pt[:, q * P:(q + 1) * P],
                                    c1[i][:, t * P:(t + 1) * P], ident)
            nc.vector.tensor_reduce(rm2p[:, 2 * t + g:2 * t + g + 1], pt,
                                    axis=AX.X, op=ALU.max, negate=True)
        nc.vector.tensor_tensor(rm2[:, t:t + 1], rm2p[:, 2 * t:2 * t + 1],
                                rm2p[:, 2 * t + 1:2 * t + 2], op=ALU.min)
        for g in range(2):
            # Z2 partial straight from PSUM; fused shift on evacuation
            scrz = sb.tile([P, 512], bf16, tag="scr", bufs=2, name=f"scrz_{j}_{t}_{g}")
            nc.scalar.activation(scrz, pts[g], ACTF.Exp, bias=rm2[:, t:t + 1],
                                 accum_out=z2p[:, 2 * t + g:2 * t + g + 1])
            nc.vector.tensor_add(epre[:, g * 512:(g + 1) * 512], pts[g],
                                 rm1b[:, g * 512:(g + 1) * 512])
        nc.vector.tensor_tensor(z2[:, t:t + 1], z2p[:, 2 * t:2 * t + 1],
                                z2p[:, 2 * t + 1:2 * t + 2], op=ALU.add)
        nc.scalar.activation(e1_t, epre, ACTF.Exp)
        _keeper(nc, ps, kc, f"kp1_{j}_{t}")

    rm2b = _bcast_rows(rm2, "rm2b", f"rm2b_{j}")

    # ---- E2T = exp(C1 - rm2[t]) + Z1 ----
    z1 = sb.tile([P, NI], f32, tag="z1", bufs=2, name=f"z1_{j}")
    e2 = st["e2"] = []
    for i in range(NI):
        e2_i = sb.tile([P, S], bf16, tag="e2", bufs=NI + 2, name=f"e2_{j}_{i}")
        e2.append(e2_i)
        epre2 = sb.tile([P, S], f32, tag="epre", bufs=2, name=f"epre2_{j}_{i}")
        nc.vector.tensor_add(epre2, c1[i], rm2b)
        nc.scalar.activation(e2_i, epre2, ACTF.Exp)
        # Z1[s] = sum_t exp(C1[s,t] - rm1[s]): ACT pass, output discarded
        scr = sb.tile([P, S], bf16, tag="scr", bufs=2, name=f"scr1_{j}_{i}")
        nc.scalar.activation(scr, c1[i], ACTF.Exp, bias=rm1[:, i:i + 1],
                             accum_out=z1[:, i:i + 1])
        _keeper(nc, ps, kc, f"kp2_{j}_{i}")

    invz1 = st["invz1"] = sb.tile([P, NI], f32, tag="invz1", bufs=2, name=f"invz1_{j}")
    invz2 = st["invz2"] = sb.tile([P, NI], f32, tag="invz2", bufs=2, name=f"invz2_{j}")
    nc.vector.reciprocal(invz1, z1)
    nc.vector.reciprocal(invz2, z2)


def _emit_p3(nc, sb, ps, st, j, m1n, m2n, outp):
    e1, e2 = st["e1"], st["e2"]
    invz1, invz2 = st["invz1"], st["invz2"]
    for c in range(NCH):
        c0 = c * CW
        r2 = sb.tile([P, NK, CW], bf16, tag="rhs", bufs=3, name=f"r2_{j}_{c}")
        r1 = sb.tile([P, NK, CW], bf16, tag="rhs", bufs=3, name=f"r1_{j}_{c}")
        nc.gpsimd.dma_start(
            out=r2, in_=m2n[j].rearrange("(k p) d -> p k d", p=P)[:, :, c0:c0 + CW])
        nc.gpsimd.dma_start(
            out=r1, in_=m1n[j].rearrange("(k p) d -> p k d", p=P)[:, :, c0:c0 + CW])

        for i in range(NI):
            for (es, rhs, gate, invz, dbase) in (
                (e1, r2, r1, invz1, 0),
                (e2, r1, r2, invz2, D),
            ):
                pav = ps.tile([P, CW], f32, tag="av", bufs=4,
                              name=f"pav{j}_{c}_{i}_{dbase}")
                for k in range(NK):
                    nc.tensor.matmul(
                        pav,
                        es[k][:, i * P:(i + 1) * P],
                        rhs[:, k, :],
                        start=(k == 0),
                        stop=(k == NK - 1),
                    )
                a_sb = sb.tile([P, CW], f32, tag="ao", bufs=4,
                               name=f"a{j}_{c}_{i}_{dbase}")
                nc.vector.scalar_tensor_tensor(
                    a_sb, pav, invz[:, i:i + 1],
                    gate[:, i, :],
                    op0=ALU.mult, op1=ALU.mult)
                nc.sync.dma_start(
                    out=outp[j, i * P:(i + 1) * P,
                             dbase + c0:dbase + c0 + CW],
                    in_=a_sb)


def _build():
    nc = bacc.Bacc("TRN2", target_bir_lowering=False, debug=False,
                   num_devices=N_CORES)
    m1n = nc.dram_tensor("m1n", [BPC, S, D], f32, kind="ExternalInput").ap()
    m2n = nc.dram_tensor("m2n", [BPC, S, D], f32, kind="ExternalInput").ap()
    m1t = nc.dram_tensor("m1t", [BPC, D, S], f32, kind="ExternalInput").ap()
    m2t = nc.dram_tensor("m2t", [BPC, D, S], f32, kind="ExternalInput").ap()
    outp = nc.dram_tensor("out", [BPC, S, 2 * D], f32, kind="ExternalOutput").ap()

    with tile.TileContext(nc) as tc:
        with tc.tile_pool(name="consts", bufs=1) as consts, \
             tc.tile_pool(name="sb", bufs=1) as sb, \
             tc.tile_pool(name="ps", bufs=1, space="PSUM") as ps:
            ident = consts.tile([P, P], f32)
            make_identity(nc, ident)
            kc = consts.tile([P, 512], f32r)
            nc.vector.memset(kc.bitcast(f32), 1.0)
            # Software-pipelined emission: PE stream becomes
            # C1(0), trans(0), C1(1), AV(0), trans(1), C1(2), AV(1), ...
            # so scores matmuls of batch j+1 fill the PE while batch j's
            # softmax runs on Vector/Scalar, and HAM stays warm. P1(j+1)
            # must be emitted after P2(j): the c1 strip slots are freed by
            # P2(j) work that sits behind P1(j+1) in the per-engine queues
            # otherwise (head-of-line deadlock).
            sts = [dict() for _ in range(BPC)]
            _emit_p1(nc, sb, ps, ident, sts[0], 0, m1t, m2t)
            for j in range(BPC):
                _emit_p2(nc, sb, ps, ident, kc, sts[j], j)
                if j + 1 < BPC:
                    _emit_p1(nc, sb, ps, ident, sts[j + 1], j + 1, m1t, m2t)
                _emit_p3(nc, sb, ps, sts[j], j, m1n, m2n, outp)
    nc.compile()
    return nc


_NC_CACHE = None


def _get_nc():
    global _NC_CACHE
    if _NC_CACHE is None:
        _NC_CACHE = _build()
    return _NC_CACHE


def kernel(mode1: np.ndarray, mode2: np.ndarray, _trace: bool = False,
           _result_box: dict | None = None) -> np.ndarray:
    mode1 = np.asarray(mode1, dtype=np.float32)
    mode2 = np.asarray(mode2, dtype=np.float32)

    m1n_all = np.ascontiguousarray(mode1.transpose(1, 0, 2))  # [B, S, D]
    m2n_all = np.ascontiguousarray(mode2.transpose(1, 0, 2))
    m1t_all = np.ascontiguousarray(mode1.transpose(1, 2, 0))  # [B, D, S]
    m2t_all = np.ascontiguousarray(mode2.transpose(1, 2, 0))

    nc = _get_nc()
    in_maps = []
    for c in range(N_CORES):
        lo, hi = c * BPC, (c + 1) * BPC
        in_maps.append({
            "m1n": m1n_all[lo:hi],
            "m2n": m2n_all[lo:hi],
            "m1t": m1t_all[lo:hi],
            "m2t": m2t_all[lo:hi],
        })

    r = None
    last_err = None
    for attempt in range(3):
        try:
            r = run_bass_kernel_spmd(nc, in_maps, list(range(N_CORES)),
                                     trace=_trace)
            break
        except Exception as e:  # transient NRT exec-unit errors recover on retry
            last_err = e
            time.sleep(2.0)
    if r is None:
        raise last_err
    if _result_box is not None:
        _result_box["result"] = r

    out = np.empty((S, B, 2 * D), dtype=np.float32)
    for c in range(N_CORES):
        res = r.results[c]["out"]  # [BPC, S, 2D]
        out[:, c * BPC:(c + 1) * BPC, :] = res.transpose(1, 0, 2)
    return out



# revision 10
# speedup vs baseline: 1.2847x; 1.2847x over previous
"""BiModalAttention Trainium2 kernel (v3 — interleaved pipeline).

Full-input contract: kernel(mode1, mode2) -> [S, B, 2D] float32.
mode1/mode2: [S=1024, B=32, D=1024] float32.

Reference computation per batch b (m1 = mode1[:, b, :], m2 = mode2[:, b, :]):
    C1 = m1 @ m2.T                  # [S, S]
    a1 = softmax_rows(C1) @ m2 * m1
    a2 = softmax_rows(C1.T) @ m1 * m2
    out[:, b, :] = concat([a1, a2], -1)

Sharding: batch dim across 8 NeuronCores (4 batch elements per core).

v3 changes vs v2 (569us baseline):
  - PE-dense interleaved emission. The PE queue never sits behind a
    softmax phase: round j emits
      A(j): C2-transpose/softmax t-loop of batch j  ||  AV-dir2 of batch j-1
      B(j): e2 i-loop of batch j                    ||  AV-dir1 of batch j
      C(j): scores matmuls of batch j+1 (PE-dense on their own)
    This removes both the ~110us of PE idle and most of the ~87us HAM
    half-clock penalty the phase-serial v2 paid (transposes don't count
    as PE activity for the HAM clock gate, so v2's softmax phases let the
    PE clock drop to 1.2GHz despite keeper matmuls).
  - Z1 accumulation moved from the e2 loop into A(j) (reads only c1+rm1),
    so invz1 is ready when B(j)'s dir-1 evacuations need it.
  - AV-dir2 evacuation split as ACT copy(scale=invz) + GpSimd gate-mult;
    dir-1 stays a single DVE scalar_tensor_tensor. Balances DVE in step A.
  - m1n/m2n stored bf16 in HBM (host-side cast; they only feed the bf16
    AV rhs/gate tiles) -> input DMA 64MB -> 48MB per core.
  - keeper matmuls dropped.
Scores stay fp32r: at N=512 moving they already run 1 cyc/row (bf16 rate)
and bf16 scores fail accuracy (softmax here is ~argmax; bf16 logit noise
flips near-tie selections: measured 6.9e-2 scale-rel error vs 2e-2 gate).
"""

import os
os.environ.setdefault("NEURON_RT_RESET_CORES", "1")
import time

import numpy as np

import concourse.bacc as bacc
import concourse.mybir as mybir
import concourse.tile as tile
from concourse.masks import make_identity
from concourse.bass_utils import run_bass_kernel_spmd

S = 1024
D = 1024
B = 32
N_CORES = 8
BPC = B // N_CORES          # batch elements per core
P = 128                     # partitions
NK = S // P                 # contraction tiles (8)
NI = S // P                 # s tiles (8)
CW = 512                    # AV d-chunk width
NCH = D // CW               # AV chunks (2)

f32 = mybir.dt.float32
f32r = mybir.dt.float32r
bf16 = mybir.dt.bfloat16
AX = mybir.AxisListType
ALU = mybir.AluOpType
ACTF = mybir.ActivationFunctionType


def _emit_t_loads(nc, sb, st, j, m1t, m2t):
    """Scores operands for batch j, d-major, f32->f32r, halved loads."""
    m1t_sb = st["m1t_sb"] = sb.tile([P, NK, S], f32r, tag="m1t", bufs=1,
                                    name=f"m1t_sb{j}")
    m2t_sb = st["m2t_sb"] = sb.tile([P, NK, S], f32r, tag="m2t", bufs=1,
                                    name=f"m2t_sb{j}")
    for lo in range(0, NK, 2):
        nc.gpsimd.dma_start(
            out=m1t_sb[:, lo:lo + 2, :],
            in_=m1t[j].rearrange("(k p) s -> p k s", p=P)[:, lo:lo + 2, :])
        nc.gpsimd.dma_start(
            out=m2t_sb[:, lo:lo + 2, :],
            in_=m2t[j].rearrange("(k p) s -> p k s", p=P)[:, lo:lo + 2, :])


def _emit_r_loads(nc, sb, st, j, m1n, m2n, c):
    """AV rhs/gate chunk c for batch j (bf16 HBM -> bf16 SBUF).

    Emission point matters: for j>=1 the pool slots being claimed are freed
    by AV-dir2(j-1) gate reads inside A(j), so this must be emitted after
    the t-iteration whose GpSimd gate-multiplies release them (t=3 for c0,
    t=7 for c1) or the scalar queue deadlocks behind the trigger's wait."""
    st.setdefault("r1", {})
    st.setdefault("r2", {})
    c0 = c * CW
    r2 = st["r2"][c] = sb.tile([P, NK, CW], bf16, tag="rhs", bufs=4,
                               name=f"r2_{j}_{c}")
    r1 = st["r1"][c] = sb.tile([P, NK, CW], bf16, tag="rhs", bufs=4,
                               name=f"r1_{j}_{c}")
    for lo in range(0, NK, 4):
        nc.scalar.dma_start(
            out=r2[:, lo:lo + 4, :],
            in_=m2n[j].rearrange("(k p) d -> p k d", p=P)[:, lo:lo + 4, c0:c0 + CW])
        nc.scalar.dma_start(
            out=r1[:, lo:lo + 4, :],
            in_=m1n[j].rearrange("(k p) d -> p k d", p=P)[:, lo:lo + 4, c0:c0 + CW])


def _emit_scores(nc, sb, ps, st, j):
    """C1 = m1 @ m2.T for batch j: 16 fp32r PSUM groups -> c1 strips + rm1."""
    m1t_sb, m2t_sb = st["m1t_sb"], st["m2t_sb"]
    c1 = st["c1"] = []
    rm1 = st["rm1"] = sb.tile([P, NI], f32, tag="rm1", bufs=2, name=f"rm1_{j}")
    for i in range(NI):
        c1_i = sb.tile([P, S], f32, tag="c1", bufs=NI, name=f"c1_{j}_{i}")
        c1.append(c1_i)
        for n in range(2):
            pc = ps.tile([P, 512], f32, tag="c", bufs=4, name=f"pc{j}_{i}_{n}")
            for k in range(NK):
                nc.tensor.matmul(
                    pc,
                    m1t_sb[:, k, i * P:(i + 1) * P],
                    m2t_sb[:, k, n * 512:(n + 1) * 512],
                    start=(k == 0),
                    stop=(k == NK - 1),
                )
            nc.scalar.copy(out=c1_i[:, n * 512:(n + 1) * 512], in_=pc)
        nc.vector.tensor_reduce(rm1[:, i:i + 1], c1_i, axis=AX.X,
                                op=ALU.max, negate=True)


def _bcast_rows(nc, sb, ps, ident, rm_cols, tag, nm):
    """RMB[t, s] = rm_cols[s] for all t (partition broadcast via PE)."""
    rmb = sb.tile([P, S], f32, tag=tag, bufs=1, name=nm)
    for g in range(2):
        pt = ps.tile([P, 512], f32, tag="c", bufs=4, name=f"{nm}_pt{g}")
        for q in range(4):
            i = g * 4 + q
            xb = sb.tile([P, P], f32, tag="xb", bufs=3, name=f"{nm}_xb{i}")
            nc.vector.tensor_copy(xb, rm_cols[:, i:i + 1].broadcast_to([P, P]))
            nc.tensor.transpose(pt[:, q * P:(q + 1) * P], xb, ident)
        nc.scalar.copy(out=rmb[:, g * 512:(g + 1) * 512], in_=pt)
    return rmb


def _emit_av_group(nc, sb, ps, st, j, es, rhs, gate, invz, i, c, dbase, outp,
                   evac):
    """One AV output tile: psum = sum_k es[k][:, i-block].T @ rhs[:, k, :];
    out = (psum * invz[i]) * gate[:, i, :]."""
    pav = ps.tile([P, CW], f32, tag="av", bufs=4,
                  name=f"pav{j}_{c}_{i}_{dbase}")
    for k in range(NK):
        nc.tensor.matmul(
            pav,
            es[k][:, i * P:(i + 1) * P],
            rhs[:, k, :],
            start=(k == 0),
            stop=(k == NK - 1),
        )
    a_sb = sb.tile([P, CW], f32, tag="ao", bufs=6,
                   name=f"a{j}_{c}_{i}_{dbase}")
    if evac == "dve":
        nc.vector.scalar_tensor_tensor(
            a_sb, pav, invz[:, i:i + 1], gate[:, i, :],
            op0=ALU.mult, op1=ALU.mult)
    else:
        # ACT scale-copy + GpSimd gate multiply (keeps DVE free in step A)
        nc.scalar.activation(a_sb, pav, ACTF.Copy, scale=invz[:, i:i + 1])
        nc.gpsimd.tensor_mul(a_sb, a_sb, gate[:, i, :])
    nc.sync.dma_start(
        out=outp[j, i * P:(i + 1) * P, dbase + c * CW:dbase + (c + 1) * CW],
        in_=a_sb)


def _emit_A(nc, sb, ps, ident, st, prev, j, jprev, outp, r_load=None):
    """t-loop of batch j (C2 strips via PE transpose -> rm2, z2, e1) with
    AV-dir2 of batch j-1 interleaved into the PE stream. Also accumulates
    z1(j) (reads only c1+rm1)."""
    c1, rm1 = st["c1"], st["rm1"]
    rm1b = _bcast_rows(nc, sb, ps, ident, rm1, "rm1b", f"rm1b_{j}")

    # AV-dir2 groups of the previous batch, c-major so r-chunk lifetimes
    # are half-pass: (c0,i0..7), (c1,i0..7); two groups per t-iteration.
    d2 = [(c, i) for c in range(NCH) for i in range(NI)] if prev else []

    e1 = st["e1"] = []
    rm2p = sb.tile([P, 2 * NK], f32, tag="rm2p", bufs=2, name=f"rm2p_{j}")
    rm2 = st["rm2"] = sb.tile([P, NK], f32, tag="rm2", bufs=2, name=f"rm2_{j}")
    z2p = sb.tile([P, 2 * NK], f32, tag="z2p", bufs=2, name=f"z2p_{j}")
    z2 = sb.tile([P, NK], f32, tag="z2", bufs=2, name=f"z2_{j}")
    z1 = sb.tile([P, NI], f32, tag="z1", bufs=2, name=f"z1_{j}")
    for t in range(NK):
        e1_t = sb.tile([P, S], bf16, tag="e1", bufs=NK, name=f"e1_{j}_{t}")
        e1.append(e1_t)
        epre = sb.tile([P, S], f32, tag="h", bufs=2, name=f"epre1_{j}_{t}")
        pts = []
        for g in range(2):
            pt = ps.tile([P, 512], f32, tag="c", bufs=4, name=f"pc2_{j}_{t}_{g}")
            pts.append(pt)
            for q in range(4):
                i = g * 4 + q
                nc.tensor.transpose(pt[:, q * P:(q + 1) * P],
                                    c1[i][:, t * P:(t + 1) * P], ident)
            nc.vector.tensor_reduce(rm2p[:, 2 * t + g:2 * t + g + 1], pt,
                                    axis=AX.X, op=ALU.max, negate=True)
        # AV-dir2(j-1): two groups keep the PE busy while DVE/ACT run softmax
        for (c, i) in d2[2 * t:2 * t + 2]:
            _emit_av_group(nc, sb, ps, prev, jprev, prev["e2"],
                           prev["r1"][c], prev["r2"][c], prev["invz2"],
                           i, c, D, outp, evac="act")
        nc.vector.tensor_tensor(rm2[:, t:t + 1], rm2p[:, 2 * t:2 * t + 1],
                                rm2p[:, 2 * t + 1:2 * t + 2], op=ALU.min)
        for g in range(2):
            # Z2 partial straight from PSUM; fused shift on evacuation
            scrz = sb.tile([P, 512], bf16, tag="scr", bufs=2,
                           name=f"scrz_{j}_{t}_{g}")
            nc.scalar.activation(scrz, pts[g], ACTF.Exp, bias=rm2[:, t:t + 1],
                                 accum_out=z2p[:, 2 * t + g:2 * t + g + 1])
            nc.vector.tensor_add(epre[:, g * 512:(g + 1) * 512], pts[g],
                                 rm1b[:, g * 512:(g + 1) * 512])
        nc.vector.tensor_tensor(z2[:, t:t + 1], z2p[:, 2 * t:2 * t + 1],
                                z2p[:, 2 * t + 1:2 * t + 2], op=ALU.add)
        nc.scalar.activation(e1_t, epre, ACTF.Exp)
        # Z1[t-th strip]: ACT exp-accumulate pass over c1[t], output discarded
        scr = sb.tile([P, S], bf16, tag="scr", bufs=2, name=f"scr1_{j}_{t}")
        nc.scalar.activation(scr, c1[t], ACTF.Exp, bias=rm1[:, t:t + 1],
                             accum_out=z1[:, t:t + 1])
        # r(j) chunk loads, placed where their pool slots have just been
        # freed (see _emit_r_loads); for j=0 they were loaded in the prologue
        if r_load is not None and t in (3, NK - 1):
            r_load(0 if t == 3 else 1)

    invz1 = st["invz1"] = sb.tile([P, NI], f32, tag="invz1", bufs=2,
                                  name=f"invz1_{j}")
    invz2 = st["invz2"] = sb.tile([P, NI], f32, tag="invz2", bufs=2,
                                  name=f"invz2_{j}")
    nc.vector.reciprocal(invz1, z1)
    nc.vector.reciprocal(invz2, z2)


def _emit_B(nc, sb, ps, ident, st, j, outp):
    """e2 i-loop of batch j with AV-dir1(j) interleaved (c-major passes)."""
    c1 = st["c1"]
    rm2b = _bcast_rows(nc, sb, ps, ident, st["rm2"], "rm2b", f"rm2b_{j}")
    e2 = st["e2"] = []
    for i in range(NI):
        e2_i = sb.tile([P, S], bf16, tag="e2", bufs=NI, name=f"e2_{j}_{i}")
        e2.append(e2_i)
        epre2 = sb.tile([P, S], f32, tag="h", bufs=2, name=f"epre2_{j}_{i}")
        nc.vector.tensor_add(epre2, c1[i], rm2b)
        nc.scalar.activation(e2_i, epre2, ACTF.Exp)
        # AV-dir1 chunk 0 rides along the i-loop
        _emit_av_group(nc, sb, ps, st, j, st["e1"], st["r2"][0], st["r1"][0],
                       st["invz1"], i, 0, 0, outp, evac="dve")
    for i in range(NI):
        _emit_av_group(nc, sb, ps, st, j, st["e1"], st["r2"][1], st["r1"][1],
                       st["invz1"], i, 1, 0, outp, evac="dve")


def _build():
    nc = bacc.Bacc("TRN2", target_bir_lowering=False, debug=False,
                   num_devices=N_CORES)
    m1n = nc.dram_tensor("m1n", [BPC, S, D], bf16, kind="ExternalInput").ap()
    m2n = nc.dram_tensor("m2n", [BPC, S, D], bf16, kind="ExternalInput").ap()
    m1t = nc.dram_tensor("m1t", [BPC, D, S], f32, kind="ExternalInput").ap()
    m2t = nc.dram_tensor("m2t", [BPC, D, S], f32, kind="ExternalInput").ap()
    outp = nc.dram_tensor("out", [BPC, S, 2 * D], f32, kind="ExternalOutput").ap()

    with tile.TileContext(nc) as tc:
        with tc.tile_pool(name="consts", bufs=1) as consts, \
             tc.tile_pool(name="sb", bufs=1) as sb, \
             tc.tile_pool(name="ps", bufs=1, space="PSUM") as ps:
            ident = consts.tile([P, P], f32)
            make_identity(nc, ident)

            sts = [dict() for _ in range(BPC)]
            _emit_t_loads(nc, sb, sts[0], 0, m1t, m2t)
            _emit_r_loads(nc, sb, sts[0], 0, m1n, m2n, 0)
            _emit_r_loads(nc, sb, sts[0], 0, m1n, m2n, 1)
            _emit_scores(nc, sb, ps, sts[0], 0)
            for j in range(BPC):
                if j + 1 < BPC:
                    _emit_t_loads(nc, sb, sts[j + 1], j + 1, m1t, m2t)
                prev = sts[j - 1] if j >= 1 else None
                r_load = ((lambda c, _j=j: _emit_r_loads(nc, sb, sts[_j], _j,
                                                         m1n, m2n, c))
                          if j >= 1 else None)
                _emit_A(nc, sb, ps, ident, sts[j], prev, j, j - 1, outp,
                        r_load=r_load)
                _emit_B(nc, sb, ps, ident, sts[j], j, outp)
                if j + 1 < BPC:
                    _emit_scores(nc, sb, ps, sts[j + 1], j + 1)
            # epilogue: AV-dir2 of the last batch
            last = sts[BPC - 1]
            for c in range(NCH):
                for i in range(NI):
                    _emit_av_group(nc, sb, ps, last, BPC - 1, last["e2"],
                                   last["r1"][c], last["r2"][c],
                                   last["invz2"], i, c, D, outp, evac="dve")
    nc.compile()
    return nc


_NC_CACHE = None


def _get_nc():
    global _NC_CACHE
    if _NC_CACHE is None:
        _NC_CACHE = _build()
    return _NC_CACHE


def kernel(mode1: np.ndarray, mode2: np.ndarray, _trace: bool = False,
           _result_box: dict | None = None) -> np.ndarray:
    import ml_dtypes
    mode1 = np.asarray(mode1, dtype=np.float32)
    mode2 = np.asarray(mode2, dtype=np.float32)

    m1n_all = np.ascontiguousarray(
        mode1.transpose(1, 0, 2)).astype(ml_dtypes.bfloat16)  # [B, S, D] bf16
    m2n_all = np.ascontiguousarray(
        mode2.transpose(1, 0, 2)).astype(ml_dtypes.bfloat16)
    m1t_all = np.ascontiguousarray(mode1.transpose(1, 2, 0))  # [B, D, S] f32
    m2t_all = np.ascontiguousarray(mode2.transpose(1, 2, 0))

    nc = _get_nc()
    in_maps = []
    for c in range(N_CORES):
        lo, hi = c * BPC, (c + 1) * BPC
        in_maps.append({
            "m1n": m1n_all[lo:hi],
            "m2n": m2n_all[lo:hi],
            "m1t": m1t_all[lo:hi],
            "m2t": m2t_all[lo:hi],
        })

    r = None
    last_err = None
    for attempt in range(3):
        try:
            r = run_bass_kernel_spmd(nc, in_maps, list(range(N_CORES)),
                                     trace=_trace)
            break
        except Exception as e:  # transient NRT exec-unit errors recover on retry
            last_err = e
            time.sleep(2.0)
    if r is None:
        raise last_err
    if _result_box is not None:
        _result_box["result"] = r

    out = np.empty((S, B, 2 * D), dtype=np.float32)
    for c in range(N_CORES):
        res = r.results[c]["out"]  # [BPC, S, 2D]
        out[:, c * BPC:(c + 1) * BPC, :] = res.transpose(1, 0, 2)
    return out
